# revision 7
# baseline (speedup 1.0000x reference)
"""AttentionPooling Trainium2 kernel.

Reference computation (per batch b):
    q   = q_emb[questions[b]]                      # (18, 128)
    qk  = (q @ x[b].T) / sqrt(128)                 # (18, 2048)
    attn= softmax(qk + log(mask))                  # masked softmax over s
    out = attn @ x[b]                              # (18, 128)

Strategy: data-parallel over batch across 8 cores (16 batches/core).
Per batch on-device:
  - load x[b] (2048,128) into SBUF as xn[p, c, d] with s = 16*p + c
    (16 chunks of 128 s-values on partitions), plus a ones column per
    chunk for the softmax denominator.
  - PE-transpose each 128x128 chunk -> xt[d, s] tile (matmul vs identity),
    PSUM->SBUF copies split between ScalarE/VectorE.
  - MM1: qkT[s_c, nq] = xt_c^T(weights) @ qT (host-gathered, pre-scaled)
  - exp on ScalarE straight out of PSUM (no max subtraction: |qk| <~ 6
    since inputs are N(0,1) and scaled by 1/sqrt(D); exp stays in fp32
    range), multiply by 0/1 mask (broadcast along nq).
  - MM2: out[nq, 0:129] accumulates attnT_c^T @ [x_c | 1] over chunks;
    column 128 is the softmax denominator.
  - normalize with reciprocal, DMA out.
"""

import math
from contextlib import ExitStack

import ml_dtypes
import numpy as np

import concourse.bass as bass
import concourse.tile as tile
from concourse import bacc, mybir
from concourse.bass_utils import run_bass_kernel_spmd
from concourse.masks import make_identity

B, S, D = 128, 2048, 128
NQ, QDIM = 18, 100
N_CORES = 8
BPC = B // N_CORES  # batches per core
C = 16              # s-chunks per batch (S = 128 * C), s = 16*p + c
CW = 130            # chunk width in xn tile: 128 data + 1 ones + 1 pad

_NC_CACHE: dict = {}


def build_nc_v2(bpc: int = BPC, reps: int = 1, stage: str = "full",
                skew: int = 1, cw: int = 129, xn_bufs: int = 4,
                xt_bufs: int = 2, mm2: str = "f32"):
    """v2: f32 HWDGE load (no cast-DMA), f32 PE transposes with free
    f32->bf16 cast in the PSUM->SBUF copy, bf16 MM1, float32r MM2 with
    the softmax denominator fused as a ones column (cw=129), and the
    MM2/normalize/store tail software-pipelined one batch behind the
    transpose/MM1 head (skew=1) so the PE never stalls on the ACT/DVE
    softmax round-trip.
    """
    f32 = mybir.dt.float32
    f32r = mybir.dt.float32r
    bf16 = mybir.dt.bfloat16

    nc = bacc.Bacc("TRN2", target_bir_lowering=False, debug=False)
    xs = nc.dram_tensor("xs", [bpc, S, D], f32, kind="ExternalInput").ap()
    qts = nc.dram_tensor("qts", [bpc, D, NQ], bf16, kind="ExternalInput").ap()
    mks = nc.dram_tensor("mks", [bpc, 128, C], f32, kind="ExternalInput").ap()
    out = nc.dram_tensor("out", [bpc, NQ, D], f32, kind="ExternalOutput").ap()

    xr = xs.rearrange("b (p c) d -> b p c d", p=128)

    with tile.TileContext(nc) as tc:
        with ExitStack() as ctx:
            singles = ctx.enter_context(tc.tile_pool(name="singles", bufs=1))
            xn_pool = ctx.enter_context(tc.tile_pool(name="xn", bufs=xn_bufs))
            xt_pool = ctx.enter_context(tc.tile_pool(name="xt", bufs=xt_bufs))
            e_pool = ctx.enter_context(tc.tile_pool(name="e", bufs=2))
            sm_pool = ctx.enter_context(tc.tile_pool(name="sm", bufs=3))
            ob_pool = ctx.enter_context(tc.tile_pool(name="ob", bufs=3))
            ps_xt_pool = ctx.enter_context(
                tc.tile_pool(name="ps_xt", bufs=4, space="PSUM")
            )
            ps_qk_pool = ctx.enter_context(
                tc.tile_pool(name="ps_qk", bufs=2, space="PSUM")
            )
            ps_o_pool = ctx.enter_context(
                tc.tile_pool(name="ps_o", bufs=2, space="PSUM")
            )

            ident = singles.tile([128, 128], f32)
            make_identity(nc, ident[:])
            qta = singles.tile([D, bpc, NQ], bf16)
            nc.sync.dma_start(out=qta[:], in_=qts.rearrange("b p n -> p b n"))
            mka = singles.tile([128, bpc, C], f32)
            nc.sync.dma_start(out=mka[:], in_=mks.rearrange("b p c -> p b c"))

            def load(b):
                xn = xn_pool.tile([128, C, cw], f32, tag="xn")
                nc.sync.dma_start(out=xn[:, :, 0:D], in_=xr[b])
                if cw > D:
                    nc.vector.memset(xn[:, :, D:cw], 1.0)
                return xn

            def zero_out(b):
                ob = ob_pool.tile([NQ, D], f32)
                nc.vector.memset(ob[:], 0.0)
                nc.scalar.dma_start(out=out[b], in_=ob[:])

            def head(b, xn):
                # transposes: xt[d, 16 chunks of 128 s], f32->bf16 in copy
                xt = xt_pool.tile([128, C * 128], bf16, tag="xt")
                for g in range(4):
                    ps_xt = ps_xt_pool.tile([128, 512], f32, tag="ps_xt")
                    for j in range(4):
                        c = 4 * g + j
                        nc.tensor.transpose(
                            ps_xt[:, j * 128 : (j + 1) * 128],
                            xn[:, c, 0:D],
                            ident[:],
                        )
                    dst = xt[:, g * 512 : (g + 1) * 512]
                    if g % 2 == 0:
                        nc.scalar.copy(dst, ps_xt[:])
                    else:
                        nc.vector.tensor_copy(dst, ps_xt[:])
                if stage == "t":
                    return None
                # MM1: qkT[s, nq] per chunk (lhsT = xt chunk, rhs = qt)
                ps_qk = ps_qk_pool.tile([128, C, NQ], f32, tag="ps_qk")
                for c in range(C):
                    nc.tensor.matmul(
                        ps_qk[:, c, :],
                        lhsT=xt[:, c * 128 : (c + 1) * 128],
                        rhs=qta[:, b, :],
                        start=True,
                        stop=True,
                    )
                if stage == "mm1":
                    return None
                e = e_pool.tile([128, C, NQ], f32, tag="e")
                nc.scalar.activation(
                    e[:], ps_qk[:], mybir.ActivationFunctionType.Exp
                )
                at_dt = bf16 if mm2 == "mixed" else f32
                at = e_pool.tile([128, C, NQ], at_dt, tag="at")
                mk_b = mka[:, b, :].unsqueeze(2).broadcast_to([128, C, NQ])
                nc.vector.tensor_mul(at[:], e[:], mk_b)
                return at

            def tail(b, xn, at):
                if stage in ("t", "mm1"):
                    zero_out(b)
                    return
                ps_o = ps_o_pool.tile([NQ, cw], f32, tag="ps_o")
                for c in range(C):
                    lhsT, rhs = at[:, c, :], xn[:, c, :]
                    if mm2 == "f32r":
                        lhsT, rhs = lhsT.bitcast(f32r), rhs.bitcast(f32r)
                    nc.tensor.matmul(
                        ps_o[:],
                        lhsT=lhsT,
                        rhs=rhs,
                        start=(c == 0),
                        stop=(c == C - 1),
                    )
                r = sm_pool.tile([NQ, 1], f32, tag="r")
                nc.vector.reciprocal(r[:], ps_o[:, D : D + 1])
                ob = ob_pool.tile([NQ, D], f32)
                nc.scalar.activation(
                    ob[:],
                    ps_o[:, 0:D],
                    mybir.ActivationFunctionType.Copy,
                    scale=r[:],
                )
                nc.scalar.dma_start(out=out[b], in_=ob[:])

            def batch_loop():
                if stage == "dma":
                    for b in range(bpc):
                        load(b)
                        zero_out(b)
                    return
                prev = None
                for b in range(bpc):
                    xn = load(b)
                    at = head(b, xn)
                    if not skew:
                        tail(b, xn, at)
                        continue
                    if prev is not None:
                        tail(*prev)
                    prev = (b, xn, at)
                if skew and prev is not None:
                    tail(*prev)

            if reps > 1:
                with tc.For_i(0, reps, 1):
                    batch_loop()
            else:
                batch_loop()

    nc.compile()
    return nc


def build_nc(compute: str = "bf16", bpc: int = BPC, reps: int = 1,
             tile_t: str = "", tile_m1: str = "", stage: str = "full",
             **kw):
    if compute == "v2":
        return build_nc_v2(bpc=bpc, reps=reps, stage=stage, **kw)
    return build_nc_v1(compute, bpc, reps, tile_t, tile_m1, stage)


def build_nc_v1(compute: str = "bf16", bpc: int = BPC, reps: int = 1,
                tile_t: str = "", tile_m1: str = "", stage: str = "full"):
    """Build the per-core bass program. compute in {'f32','bf16'}.

    reps > 1 wraps the whole batch loop in a hardware For_i that redoes the
    same work `reps` times (same data, same output) — benchmarking only.

    tile_t / tile_m1: column-tiling mode for the transposes / QK matmuls:
    "" (single full-width op), "2x64" (two 64-col tiles at col groups 0/64),
    "4x32" (four 32-col tiles — quadrant 3 hangs cayman, do not use).
    Splitting loads the stationary weights through parallel XBUSes.
    """

    def col_splits(mode):
        if mode == "2x64":
            return [(0, 64), (64, 64)]
        if mode == "4x32":
            return [(0, 32), (32, 32), (64, 32), (96, 32)]
        if mode == "3t":
            return [(0, 32), (32, 32), (64, 64)]
        return [(0, 128)]
    dt = mybir.dt.bfloat16 if compute == "bf16" else mybir.dt.float32
    f32 = mybir.dt.float32
    cast_load = compute == "bf16"

    nc = bacc.Bacc("TRN2", target_bir_lowering=False, debug=False)
    xs = nc.dram_tensor("xs", [bpc, S, D], f32, kind="ExternalInput").ap()
    qts = nc.dram_tensor("qts", [bpc, D, NQ], dt, kind="ExternalInput").ap()
    mks = nc.dram_tensor("mks", [bpc, 128, C], dt, kind="ExternalInput").ap()
    out = nc.dram_tensor("out", [bpc, NQ, D], f32, kind="ExternalOutput").ap()

    xr = xs.rearrange("b (p c) d -> b p c d", p=128)

    with tile.TileContext(nc) as tc:
        with ExitStack() as ctx:
            singles = ctx.enter_context(tc.tile_pool(name="singles", bufs=1))
            xn_pool = ctx.enter_context(tc.tile_pool(name="xn", bufs=3))
            xt_pool = ctx.enter_context(tc.tile_pool(name="xt", bufs=2))
            sm_pool = ctx.enter_context(tc.tile_pool(name="sm", bufs=3))
            e_pool = ctx.enter_context(tc.tile_pool(name="e", bufs=2))
            ob_pool = ctx.enter_context(tc.tile_pool(name="ob", bufs=3))
            ps_xt_pool = ctx.enter_context(
                tc.tile_pool(name="ps_xt", bufs=4, space="PSUM")
            )
            ps_qk_pool = ctx.enter_context(
                tc.tile_pool(name="ps_qk", bufs=2, space="PSUM")
            )
            ps_o_pool = ctx.enter_context(
                tc.tile_pool(name="ps_o", bufs=2, space="PSUM")
            )

            ident = singles.tile([128, 128], dt)
            make_identity(nc, ident[:])

            # all batches' qT and mask in one DMA each (tiny)
            qta = singles.tile([D, bpc, NQ], dt)
            nc.sync.dma_start(out=qta[:], in_=qts.rearrange("b p n -> p b n"))
            mka = singles.tile([128, bpc, C], dt)
            nc.sync.dma_start(out=mka[:], in_=mks.rearrange("b p c -> p b c"))

            def body(b):
                # ---- load x[b]: s=16p+c chunk layout, f32->dt cast in DMA
                xn = xn_pool.tile([128, C, CW], dt)
                eng = nc.gpsimd if cast_load else nc.sync
                eng.dma_start(out=xn[:, :, 0:D], in_=xr[b])
                nc.vector.memset(xn[:, :, D : D + 1], 1.0)

                qt = qta[:, b, :]
                mk = mka[:, b, :]

                if stage == "dma":
                    ob = ob_pool.tile([NQ, D], f32)
                    nc.vector.memset(ob[:], 0.0)
                    nc.sync.dma_start(out=out[b], in_=ob[:])
                    return

                # ---- transpose x chunks: xt[d, 16 chunks of 128 s]
                xt = xt_pool.tile([128, C * 128], dt)
                for g in range(4):
                    ps_xt = ps_xt_pool.tile([128, 512], dt)
                    for j in range(4):
                        c = 4 * g + j
                        dst_ps = ps_xt[:, j * 128 : (j + 1) * 128]
                        for off, w in col_splits(tile_t):
                            kw = {} if w == D else {"tile_position": (0, off)}
                            nc.tensor.transpose(
                                dst_ps[off : off + w, :],
                                xn[:, c, off : off + w],
                                ident[:],
                                **kw,
                            )
                    dst = xt[:, g * 512 : (g + 1) * 512]
                    if g % 2 == 0:
                        nc.scalar.copy(dst, ps_xt[:])
                    else:
                        nc.vector.tensor_copy(dst, ps_xt[:])

                if stage == "t":
                    ob = ob_pool.tile([NQ, D], f32)
                    nc.vector.memset(ob[:], 0.0)
                    nc.sync.dma_start(out=out[b], in_=ob[:])
                    return

                # ---- MM1: qkT[s, nq] per chunk (lhsT = xT_c weights)
                ps_qk = ps_qk_pool.tile([128, C, NQ], f32)
                for c in range(C):
                    for off, w in col_splits(tile_m1):
                        kw = {} if w == D else {"tile_position": (0, off)}
                        nc.tensor.matmul(
                            ps_qk[off : off + w, c, :],
                            lhsT=xt[:, c * 128 + off : c * 128 + off + w],
                            rhs=qt,
                            start=True,
                            stop=True,
                            **kw,
                        )

                if stage == "mm1":
                    ob = ob_pool.tile([NQ, D], f32)
                    nc.vector.memset(ob[:], 0.0)
                    nc.sync.dma_start(out=out[b], in_=ob[:])
                    return

                # ---- softmax numerator: exp, then mask (0/1) broadcast
                e = e_pool.tile([128, C, NQ], dt, tag="e")
                nc.scalar.activation(e[:], ps_qk[:], mybir.ActivationFunctionType.Exp)
                at = e_pool.tile([128, C, NQ], dt, tag="at")
                mk_b = mk.unsqueeze(2).broadcast_to([128, C, NQ])
                nc.vector.tensor_mul(at[:], e[:], mk_b)

                # ---- MM2: accumulate attnT_c^T @ [x_c | 1] over chunks
                ps_o = ps_o_pool.tile([NQ, D + 1], f32)
                for c in range(C):
                    nc.tensor.matmul(
                        ps_o[:],
                        lhsT=at[:, c, :],
                        rhs=xn[:, c, 0 : D + 1],
                        start=(c == 0),
                        stop=(c == C - 1),
                    )

                # ---- normalize and store
                r = sm_pool.tile([NQ, 1], f32, tag="r")
                nc.vector.reciprocal(r[:], ps_o[:, D : D + 1])
                ob = ob_pool.tile([NQ, D], f32)
                nc.scalar.activation(
                    ob[:],
                    ps_o[:, 0:D],
                    mybir.ActivationFunctionType.Copy,
                    scale=r[:],
                )
                nc.sync.dma_start(out=out[b], in_=ob[:])

            if reps > 1:
                with tc.For_i(0, reps, 1):
                    for b in range(bpc):
                        body(b)
            else:
                for b in range(bpc):
                    body(b)

    nc.compile()
    return nc


def _get_nc(compute: str = "bf16", bpc: int = BPC):
    key = (compute, bpc)
    if key not in _NC_CACHE:
        _NC_CACHE[key] = build_nc(compute, bpc)
    return _NC_CACHE[key]


def prep_inputs(x, q_emb, questions, mask, compute: str = "bf16"):
    """Host-side prep: gather+scale+transpose the tiny q table, reshape mask."""
    q_emb = np.asarray(q_emb, dtype=np.float32)
    questions = np.asarray(questions)
    mask = np.asarray(mask)
    if compute == "v2":
        q_dt, m_dt = ml_dtypes.bfloat16, np.float32
    else:
        np_dt = ml_dtypes.bfloat16 if compute == "bf16" else np.float32
        q_dt = m_dt = np_dt
    scale = 1.0 / math.sqrt(D)
    q = (q_emb * scale)[questions]                          # (B, NQ, D)
    qT = np.ascontiguousarray(q.transpose(0, 2, 1)).astype(q_dt)  # (B, D, NQ)
    mk = np.ascontiguousarray(mask.astype(m_dt).reshape(B, 128, C))  # s = 16p+c
    return qT, mk


def kernel(x, q_emb, questions, mask, compute: str = "bf16"):
    nc = _get_nc(compute)
    qT, mk = prep_inputs(x, q_emb, questions, mask, compute)
    x = np.ascontiguousarray(np.asarray(x), dtype=np.float32)

    in_maps = []
    for k in range(N_CORES):
        sl = slice(k * BPC, (k + 1) * BPC)
        in_maps.append({"xs": x[sl], "qts": qT[sl], "mks": mk[sl]})

    res = run_bass_kernel_spmd(nc, in_maps, core_ids=list(range(N_CORES)))
    outs = np.concatenate([res.results[k]["out"] for k in range(N_CORES)], axis=0)
    return np.ascontiguousarray(outs, dtype=np.float32)


if __name__ == "__main__":
    rng = np.random.default_rng(0)
    x = rng.standard_normal((B, S, D), dtype=np.float32)
    q_emb = rng.standard_normal((QDIM, D), dtype=np.float32)
    questions = rng.integers(0, QDIM, size=(B, NQ), dtype=np.int32)
    mask = rng.integers(0, 2, size=(B, S), dtype=np.int32)
    out = kernel(x, q_emb, questions, mask)
    print(out.shape, out.dtype)



# revision 14
# speedup vs baseline: 1.5039x; 1.5039x over previous
"""AttentionPooling Trainium2 kernel.

Reference computation (per batch b):
    q   = q_emb[questions[b]]                      # (18, 128)
    qk  = (q @ x[b].T) / sqrt(128)                 # (18, 2048)
    attn= softmax(qk + log(mask))                  # masked softmax over s
    out = attn @ x[b]                              # (18, 128)

Strategy: data-parallel over batch across 8 cores (16 batches/core).
Per batch on-device:
  - load x[b] (2048,128) into SBUF as xn[p, c, d] with s = 16*p + c
    (16 chunks of 128 s-values on partitions), plus a ones column per
    chunk for the softmax denominator.
  - PE-transpose each 128x128 chunk -> xt[d, s] tile (matmul vs identity),
    PSUM->SBUF copies split between ScalarE/VectorE.
  - MM1: qkT[s_c, nq] = xt_c^T(weights) @ qT (host-gathered, pre-scaled)
  - exp on ScalarE straight out of PSUM (no max subtraction: |qk| <~ 6
    since inputs are N(0,1) and scaled by 1/sqrt(D); exp stays in fp32
    range), multiply by 0/1 mask (broadcast along nq).
  - MM2: out[nq, 0:129] accumulates attnT_c^T @ [x_c | 1] over chunks;
    column 128 is the softmax denominator.
  - normalize with reciprocal, DMA out.
"""

import math
from contextlib import ExitStack

import ml_dtypes
import numpy as np

import concourse.bass as bass
import concourse.tile as tile
from concourse import bacc, mybir
from concourse.bass_utils import run_bass_kernel_spmd
from concourse.masks import make_identity

B, S, D = 128, 2048, 128
NQ, QDIM = 18, 100
N_CORES = 8
BPC = B // N_CORES  # batches per core
C = 16              # s-chunks per batch (S = 128 * C), s = 16*p + c
CW = 130            # chunk width in xn tile: 128 data + 1 ones + 1 pad

_NC_CACHE: dict = {}


def build_nc_v2(bpc: int = BPC, reps: int = 1, stage: str = "full",
                skew: int = 1, cw: int = 129, xn_bufs: int = 4,
                xt_bufs: int = 2, mm2: str = "f32", load: str = "f32"):
    """v2: f32 HWDGE load (no cast-DMA), f32 PE transposes with free
    f32->bf16 cast in the PSUM->SBUF copy, bf16 MM1, float32r MM2 with
    the softmax denominator fused as a ones column (cw=129), and the
    MM2/normalize/store tail software-pipelined one batch behind the
    transpose/MM1 head (skew=1) so the PE never stalls on the ACT/DVE
    softmax round-trip.
    """
    f32 = mybir.dt.float32
    f32r = mybir.dt.float32r
    bf16 = mybir.dt.bfloat16
    # xd: dtype of x in SBUF. load="bf16" casts f32->bf16 in the DMA
    # (SWDGE via gpsimd); MM2 then runs in bf16 and mm2 is ignored.
    xd = bf16 if load == "bf16" else f32

    nc = bacc.Bacc("TRN2", target_bir_lowering=False, debug=False)
    xs = nc.dram_tensor("xs", [bpc, S, D], f32, kind="ExternalInput").ap()
    qts = nc.dram_tensor("qts", [bpc, D, NQ], bf16, kind="ExternalInput").ap()
    mks = nc.dram_tensor("mks", [bpc, 128, C], f32, kind="ExternalInput").ap()
    out = nc.dram_tensor("out", [bpc, NQ, D], f32, kind="ExternalOutput").ap()

    xr = xs.rearrange("b (p c) d -> b p c d", p=128)

    with tile.TileContext(nc) as tc:
        with ExitStack() as ctx:
            singles = ctx.enter_context(tc.tile_pool(name="singles", bufs=1))
            xn_pool = ctx.enter_context(tc.tile_pool(name="xn", bufs=xn_bufs))
            xt_pool = ctx.enter_context(tc.tile_pool(name="xt", bufs=xt_bufs))
            e_pool = ctx.enter_context(tc.tile_pool(name="e", bufs=2))
            sm_pool = ctx.enter_context(tc.tile_pool(name="sm", bufs=3))
            ob_pool = ctx.enter_context(tc.tile_pool(name="ob", bufs=3))
            ps_xt_pool = ctx.enter_context(
                tc.tile_pool(name="ps_xt", bufs=4, space="PSUM")
            )
            ps_qk_pool = ctx.enter_context(
                tc.tile_pool(name="ps_qk", bufs=2, space="PSUM")
            )
            ps_o_pool = ctx.enter_context(
                tc.tile_pool(name="ps_o", bufs=2, space="PSUM")
            )

            ident = singles.tile([128, 128], xd)
            make_identity(nc, ident[:])
            qta = singles.tile([D, bpc, NQ], bf16)
            nc.sync.dma_start(out=qta[:], in_=qts.rearrange("b p n -> p b n"))
            mka = singles.tile([128, bpc, C], f32)
            nc.sync.dma_start(out=mka[:], in_=mks.rearrange("b p c -> p b c"))

            def load_x(b):
                xn = xn_pool.tile([128, C, cw], xd, tag="xn")
                eng = nc.gpsimd if load == "bf16" else nc.sync
                eng.dma_start(out=xn[:, :, 0:D], in_=xr[b])
                if cw > D:
                    nc.vector.memset(xn[:, :, D:cw], 1.0)
                return xn

            def zero_out(b):
                ob = ob_pool.tile([NQ, D], f32)
                nc.vector.memset(ob[:], 0.0)
                nc.scalar.dma_start(out=out[b], in_=ob[:])

            def head(b, xn):
                # transposes: xt[d, 16 chunks of 128 s], f32->bf16 in copy
                xt = xt_pool.tile([128, C * 128], bf16, tag="xt")
                for g in range(4):
                    ps_xt = ps_xt_pool.tile([128, 512], xd, tag="ps_xt")
                    for j in range(4):
                        c = 4 * g + j
                        nc.tensor.transpose(
                            ps_xt[:, j * 128 : (j + 1) * 128],
                            xn[:, c, 0:D],
                            ident[:],
                        )
                    dst = xt[:, g * 512 : (g + 1) * 512]
                    if g % 2 == 0:
                        nc.scalar.copy(dst, ps_xt[:])
                    else:
                        nc.vector.tensor_copy(dst, ps_xt[:])
                if stage == "t":
                    return None
                # MM1: qkT[s, nq] per chunk (lhsT = xt chunk, rhs = qt)
                ps_qk = ps_qk_pool.tile([128, C, NQ], f32, tag="ps_qk")
                for c in range(C):
                    nc.tensor.matmul(
                        ps_qk[:, c, :],
                        lhsT=xt[:, c * 128 : (c + 1) * 128],
                        rhs=qta[:, b, :],
                        start=True,
                        stop=True,
                    )
                if stage == "mm1":
                    return None
                e = e_pool.tile([128, C, NQ], f32, tag="e")
                nc.scalar.activation(
                    e[:], ps_qk[:], mybir.ActivationFunctionType.Exp
                )
                at_dt = bf16 if (mm2 == "mixed" or load == "bf16") else f32
                at = e_pool.tile([128, C, NQ], at_dt, tag="at")
                mk_b = mka[:, b, :].unsqueeze(2).broadcast_to([128, C, NQ])
                nc.vector.tensor_mul(at[:], e[:], mk_b)
                return at

            def tail(b, xn, at):
                if stage in ("t", "mm1"):
                    zero_out(b)
                    return
                ps_o = ps_o_pool.tile([NQ, cw], f32, tag="ps_o")
                for c in range(C):
                    lhsT, rhs = at[:, c, :], xn[:, c, :]
                    if mm2 == "f32r" and load != "bf16":
                        lhsT, rhs = lhsT.bitcast(f32r), rhs.bitcast(f32r)
                    nc.tensor.matmul(
                        ps_o[:],
                        lhsT=lhsT,
                        rhs=rhs,
                        start=(c == 0),
                        stop=(c == C - 1),
                    )
                r = sm_pool.tile([NQ, 1], f32, tag="r")
                nc.vector.reciprocal(r[:], ps_o[:, D : D + 1])
                ob = ob_pool.tile([NQ, D], f32)
                nc.scalar.activation(
                    ob[:],
                    ps_o[:, 0:D],
                    mybir.ActivationFunctionType.Copy,
                    scale=r[:],
                )
                nc.scalar.dma_start(out=out[b], in_=ob[:])

            def batch_loop():
                if stage == "dma":
                    for b in range(bpc):
                        load_x(b)
                        zero_out(b)
                    return
                prev = None
                for b in range(bpc):
                    xn = load_x(b)
                    at = head(b, xn)
                    if not skew:
                        tail(b, xn, at)
                        continue
                    if prev is not None:
                        tail(*prev)
                    prev = (b, xn, at)
                if skew and prev is not None:
                    tail(*prev)

            if reps > 1:
                with tc.For_i(0, reps, 1):
                    batch_loop()
            else:
                batch_loop()

    nc.compile()
    return nc


def build_nc(compute: str = "bf16", bpc: int = BPC, reps: int = 1,
             tile_t: str = "", tile_m1: str = "", stage: str = "full",
             **kw):
    if compute == "v2":
        return build_nc_v2(bpc=bpc, reps=reps, stage=stage, **kw)
    return build_nc_v1(compute, bpc, reps, tile_t, tile_m1, stage)


def build_nc_v1(compute: str = "bf16", bpc: int = BPC, reps: int = 1,
                tile_t: str = "", tile_m1: str = "", stage: str = "full"):
    """Build the per-core bass program. compute in {'f32','bf16'}.

    reps > 1 wraps the whole batch loop in a hardware For_i that redoes the
    same work `reps` times (same data, same output) — benchmarking only.

    tile_t / tile_m1: column-tiling mode for the transposes / QK matmuls:
    "" (single full-width op), "2x64" (two 64-col tiles at col groups 0/64),
    "4x32" (four 32-col tiles — quadrant 3 hangs cayman, do not use).
    Splitting loads the stationary weights through parallel XBUSes.
    """

    def col_splits(mode):
        if mode == "2x64":
            return [(0, 64), (64, 64)]
        if mode == "4x32":
            return [(0, 32), (32, 32), (64, 32), (96, 32)]
        if mode == "3t":
            return [(0, 32), (32, 32), (64, 64)]
        return [(0, 128)]
    dt = mybir.dt.bfloat16 if compute == "bf16" else mybir.dt.float32
    f32 = mybir.dt.float32
    cast_load = compute == "bf16"

    nc = bacc.Bacc("TRN2", target_bir_lowering=False, debug=False)
    xs = nc.dram_tensor("xs", [bpc, S, D], f32, kind="ExternalInput").ap()
    qts = nc.dram_tensor("qts", [bpc, D, NQ], dt, kind="ExternalInput").ap()
    mks = nc.dram_tensor("mks", [bpc, 128, C], dt, kind="ExternalInput").ap()
    out = nc.dram_tensor("out", [bpc, NQ, D], f32, kind="ExternalOutput").ap()

    xr = xs.rearrange("b (p c) d -> b p c d", p=128)

    with tile.TileContext(nc) as tc:
        with ExitStack() as ctx:
            singles = ctx.enter_context(tc.tile_pool(name="singles", bufs=1))
            xn_pool = ctx.enter_context(tc.tile_pool(name="xn", bufs=3))
            xt_pool = ctx.enter_context(tc.tile_pool(name="xt", bufs=2))
            sm_pool = ctx.enter_context(tc.tile_pool(name="sm", bufs=3))
            e_pool = ctx.enter_context(tc.tile_pool(name="e", bufs=2))
            ob_pool = ctx.enter_context(tc.tile_pool(name="ob", bufs=3))
            ps_xt_pool = ctx.enter_context(
                tc.tile_pool(name="ps_xt", bufs=4, space="PSUM")
            )
            ps_qk_pool = ctx.enter_context(
                tc.tile_pool(name="ps_qk", bufs=2, space="PSUM")
            )
            ps_o_pool = ctx.enter_context(
                tc.tile_pool(name="ps_o", bufs=2, space="PSUM")
            )

            ident = singles.tile([128, 128], dt)
            make_identity(nc, ident[:])

            # all batches' qT and mask in one DMA each (tiny)
            qta = singles.tile([D, bpc, NQ], dt)
            nc.sync.dma_start(out=qta[:], in_=qts.rearrange("b p n -> p b n"))
            mka = singles.tile([128, bpc, C], dt)
            nc.sync.dma_start(out=mka[:], in_=mks.rearrange("b p c -> p b c"))

            def body(b):
                # ---- load x[b]: s=16p+c chunk layout, f32->dt cast in DMA
                xn = xn_pool.tile([128, C, CW], dt)
                eng = nc.gpsimd if cast_load else nc.sync
                eng.dma_start(out=xn[:, :, 0:D], in_=xr[b])
                nc.vector.memset(xn[:, :, D : D + 1], 1.0)

                qt = qta[:, b, :]
                mk = mka[:, b, :]

                if stage == "dma":
                    ob = ob_pool.tile([NQ, D], f32)
                    nc.vector.memset(ob[:], 0.0)
                    nc.sync.dma_start(out=out[b], in_=ob[:])
                    return

                # ---- transpose x chunks: xt[d, 16 chunks of 128 s]
                xt = xt_pool.tile([128, C * 128], dt)
                for g in range(4):
                    ps_xt = ps_xt_pool.tile([128, 512], dt)
                    for j in range(4):
                        c = 4 * g + j
                        dst_ps = ps_xt[:, j * 128 : (j + 1) * 128]
                        for off, w in col_splits(tile_t):
                            kw = {} if w == D else {"tile_position": (0, off)}
                            nc.tensor.transpose(
                                dst_ps[off : off + w, :],
                                xn[:, c, off : off + w],
                                ident[:],
                                **kw,
                            )
                    dst = xt[:, g * 512 : (g + 1) * 512]
                    if g % 2 == 0:
                        nc.scalar.copy(dst, ps_xt[:])
                    else:
                        nc.vector.tensor_copy(dst, ps_xt[:])

                if stage == "t":
                    ob = ob_pool.tile([NQ, D], f32)
                    nc.vector.memset(ob[:], 0.0)
                    nc.sync.dma_start(out=out[b], in_=ob[:])
                    return

                # ---- MM1: qkT[s, nq] per chunk (lhsT = xT_c weights)
                ps_qk = ps_qk_pool.tile([128, C, NQ], f32)
                for c in range(C):
                    for off, w in col_splits(tile_m1):
                        kw = {} if w == D else {"tile_position": (0, off)}
                        nc.tensor.matmul(
                            ps_qk[off : off + w, c, :],
                            lhsT=xt[:, c * 128 + off : c * 128 + off + w],
                            rhs=qt,
                            start=True,
                            stop=True,
                            **kw,
                        )

                if stage == "mm1":
                    ob = ob_pool.tile([NQ, D], f32)
                    nc.vector.memset(ob[:], 0.0)
                    nc.sync.dma_start(out=out[b], in_=ob[:])
                    return

                # ---- softmax numerator: exp, then mask (0/1) broadcast
                e = e_pool.tile([128, C, NQ], dt, tag="e")
                nc.scalar.activation(e[:], ps_qk[:], mybir.ActivationFunctionType.Exp)
                at = e_pool.tile([128, C, NQ], dt, tag="at")
                mk_b = mk.unsqueeze(2).broadcast_to([128, C, NQ])
                nc.vector.tensor_mul(at[:], e[:], mk_b)

                # ---- MM2: accumulate attnT_c^T @ [x_c | 1] over chunks
                ps_o = ps_o_pool.tile([NQ, D + 1], f32)
                for c in range(C):
                    nc.tensor.matmul(
                        ps_o[:],
                        lhsT=at[:, c, :],
                        rhs=xn[:, c, 0 : D + 1],
                        start=(c == 0),
                        stop=(c == C - 1),
                    )

                # ---- normalize and store
                r = sm_pool.tile([NQ, 1], f32, tag="r")
                nc.vector.reciprocal(r[:], ps_o[:, D : D + 1])
                ob = ob_pool.tile([NQ, D], f32)
                nc.scalar.activation(
                    ob[:],
                    ps_o[:, 0:D],
                    mybir.ActivationFunctionType.Copy,
                    scale=r[:],
                )
                nc.sync.dma_start(out=out[b], in_=ob[:])

            if reps > 1:
                with tc.For_i(0, reps, 1):
                    for b in range(bpc):
                        body(b)
            else:
                for b in range(bpc):
                    body(b)

    nc.compile()
    return nc


def _get_nc(compute: str = "bf16", bpc: int = BPC):
    key = (compute, bpc)
    if key not in _NC_CACHE:
        _NC_CACHE[key] = build_nc(compute, bpc)
    return _NC_CACHE[key]


def prep_inputs(x, q_emb, questions, mask, compute: str = "bf16"):
    """Host-side prep: gather+scale+transpose the tiny q table, reshape mask."""
    q_emb = np.asarray(q_emb, dtype=np.float32)
    questions = np.asarray(questions)
    mask = np.asarray(mask)
    if compute == "v2":
        q_dt, m_dt = ml_dtypes.bfloat16, np.float32
    else:
        np_dt = ml_dtypes.bfloat16 if compute == "bf16" else np.float32
        q_dt = m_dt = np_dt
    scale = 1.0 / math.sqrt(D)
    q = (q_emb * scale)[questions]                          # (B, NQ, D)
    qT = np.ascontiguousarray(q.transpose(0, 2, 1)).astype(q_dt)  # (B, D, NQ)
    mk = np.ascontiguousarray(mask.astype(m_dt).reshape(B, 128, C))  # s = 16p+c
    return qT, mk


def kernel(x, q_emb, questions, mask, compute: str = "bf16"):
    nc = _get_nc(compute)
    qT, mk = prep_inputs(x, q_emb, questions, mask, compute)
    x = np.ascontiguousarray(np.asarray(x), dtype=np.float32)

    in_maps = []
    for k in range(N_CORES):
        sl = slice(k * BPC, (k + 1) * BPC)
        in_maps.append({"xs": x[sl], "qts": qT[sl], "mks": mk[sl]})

    res = run_bass_kernel_spmd(nc, in_maps, core_ids=list(range(N_CORES)))
    outs = np.concatenate([res.results[k]["out"] for k in range(N_CORES)], axis=0)
    return np.ascontiguousarray(outs, dtype=np.float32)


if __name__ == "__main__":
    rng = np.random.default_rng(0)
    x = rng.standard_normal((B, S, D), dtype=np.float32)
    q_emb = rng.standard_normal((QDIM, D), dtype=np.float32)
    questions = rng.integers(0, QDIM, size=(B, NQ), dtype=np.int32)
    mask = rng.integers(0, 2, size=(B, S), dtype=np.int32)
    out = kernel(x, q_emb, questions, mask)
    print(out.shape, out.dtype)



# revision 48
# speedup vs baseline: 1.6007x; 1.0643x over previous
"""AttentionPooling Trainium2 kernel.

Reference computation (per batch b):
    q   = q_emb[questions[b]]                      # (18, 128)
    qk  = (q @ x[b].T) / sqrt(128)                 # (18, 2048)
    attn= softmax(qk + log(mask))                  # masked softmax over s
    out = attn @ x[b]                              # (18, 128)

Strategy: data-parallel over batch across 8 cores (16 batches/core).
Per batch on-device:
  - load x[b] (2048,128) into SBUF as xn[p, c, d] with s = 16*p + c
    (16 chunks of 128 s-values on partitions), plus a ones column per
    chunk for the softmax denominator.
  - PE-transpose each 128x128 chunk -> xt[d, s] tile (matmul vs identity),
    PSUM->SBUF copies split between ScalarE/VectorE.
  - MM1: qkT[s_c, nq] = xt_c^T(weights) @ qT (host-gathered, pre-scaled)
  - exp on ScalarE straight out of PSUM (no max subtraction: |qk| <~ 6
    since inputs are N(0,1) and scaled by 1/sqrt(D); exp stays in fp32
    range), multiply by 0/1 mask (broadcast along nq).
  - MM2: out[nq, 0:129] accumulates attnT_c^T @ [x_c | 1] over chunks;
    column 128 is the softmax denominator.
  - normalize with reciprocal, DMA out.
"""

import math
from contextlib import ExitStack

import ml_dtypes
import numpy as np

import concourse.bass as bass
import concourse.tile as tile
from concourse import bacc, mybir
from concourse.bass_utils import run_bass_kernel_spmd
from concourse.masks import make_identity

B, S, D = 128, 2048, 128
NQ, QDIM = 18, 100
N_CORES = 8
BPC = B // N_CORES  # batches per core
C = 16              # s-chunks per batch (S = 128 * C), s = 16*p + c
CW = 130            # chunk width in xn tile: 128 data + 1 ones + 1 pad

_NC_CACHE: dict = {}


def build_nc_v2(bpc: int = BPC, reps: int = 1, stage: str = "full",
                skew: int = 1, cw: int = 129, xn_bufs: int = 4,
                xt_bufs: int = 2, mm2: str = "f32", load: str = "f32",
                tile_t: str = "", tile_m1: str = "", fine: int = 0,
                dma_split: int = 1, ps_xt_bufs: int = 4, ps_qk_bufs: int = 2,
                ps_o_bufs: int = 2, e_bufs: int = 2, mm2p: int = 0,
                t_as_mm: int = 0, pc_dve: int = 0, pair: int = 0,
                ob_bufs: int = 3, sm_bufs: int = 3, tgroup: int = 4,
                ones_once: int = 0):
    def col_splits(mode):
        if mode == "2x64":
            return [(0, 64), (64, 64)]
        if mode == "3t":
            return [(0, 32), (32, 32), (64, 64)]
        return [(0, 128)]
    """v2: f32 HWDGE load (no cast-DMA), f32 PE transposes with free
    f32->bf16 cast in the PSUM->SBUF copy, bf16 MM1, float32r MM2 with
    the softmax denominator fused as a ones column (cw=129), and the
    MM2/normalize/store tail software-pipelined one batch behind the
    transpose/MM1 head (skew=1) so the PE never stalls on the ACT/DVE
    softmax round-trip.
    """
    f32 = mybir.dt.float32
    f32r = mybir.dt.float32r
    bf16 = mybir.dt.bfloat16
    # xd: dtype of x in SBUF. load="bf16" casts f32->bf16 in the DMA
    # (SWDGE via gpsimd); MM2 then runs in bf16 and mm2 is ignored.
    xd = bf16 if load == "bf16" else f32

    nc = bacc.Bacc("TRN2", target_bir_lowering=False, debug=False)
    xs = nc.dram_tensor("xs", [bpc, S, D], f32, kind="ExternalInput").ap()
    qts = nc.dram_tensor("qts", [bpc, D, NQ], bf16, kind="ExternalInput").ap()
    mks = nc.dram_tensor("mks", [bpc, 128, C], f32, kind="ExternalInput").ap()
    sel = None
    if mm2p:
        sel = nc.dram_tensor(
            "sel", [128, NQ], bf16, kind="ExternalInput"
        ).ap()
    out = nc.dram_tensor("out", [bpc, NQ, D], f32, kind="ExternalOutput").ap()

    xr = xs.rearrange("b (p c) d -> b p c d", p=128)

    with tile.TileContext(nc) as tc:
        with ExitStack() as ctx:
            singles = ctx.enter_context(tc.tile_pool(name="singles", bufs=1))
            xn_pool = ctx.enter_context(tc.tile_pool(name="xn", bufs=xn_bufs))
            xt_pool = ctx.enter_context(tc.tile_pool(name="xt", bufs=xt_bufs))
            e_pool = ctx.enter_context(tc.tile_pool(name="e", bufs=e_bufs))
            sm_pool = ctx.enter_context(tc.tile_pool(name="sm", bufs=sm_bufs))
            ob_pool = ctx.enter_context(tc.tile_pool(name="ob", bufs=ob_bufs))
            ps_xt_pool = ctx.enter_context(
                tc.tile_pool(name="ps_xt", bufs=ps_xt_bufs, space="PSUM")
            )
            ps_qk_pool = ctx.enter_context(
                tc.tile_pool(name="ps_qk", bufs=ps_qk_bufs, space="PSUM")
            )
            ps_o_pool = ctx.enter_context(
                tc.tile_pool(name="ps_o", bufs=ps_o_bufs, space="PSUM")
            )

            ident = singles.tile([128, 128], xd)
            make_identity(nc, ident[:])
            qta = singles.tile([D, bpc, NQ], bf16)
            nc.sync.dma_start(out=qta[:], in_=qts.rearrange("b p n -> p b n"))
            mka = singles.tile([128, bpc, C], f32)
            nc.sync.dma_start(out=mka[:], in_=mks.rearrange("b p c -> p b c"))
            selt = None
            if mm2p:
                selt = singles.tile([128, NQ], bf16)
                nc.sync.dma_start(out=selt[:], in_=sel)

            def load_x(b):
                xn = xn_pool.tile([128, C, cw], xd, tag="xn")
                if not stage.startswith("nodma"):
                    eng = nc.gpsimd if load == "bf16" else nc.sync
                    cs = C // dma_split
                    for k in range(dma_split):
                        eng.dma_start(
                            out=xn[:, k * cs : (k + 1) * cs, 0:D],
                            in_=xr[b][:, k * cs : (k + 1) * cs, :],
                        )
                if cw > D and (not ones_once or b < xn_bufs):
                    # pool buffers keep their ones column across rotations;
                    # the DMA only ever writes cols 0:D
                    nc.vector.memset(xn[:, :, D:cw], 1.0)
                return xn

            xr2 = xs.rearrange(
                "(h two) (p c) d -> h p two c d", two=2, p=128
            )

            def load_pair(h):
                xn2 = xn_pool.tile([128, 2, C, cw], xd, tag="xn")
                if not stage.startswith("nodma"):
                    eng = nc.gpsimd if load == "bf16" else nc.sync
                    eng.dma_start(out=xn2[:, :, :, 0:D], in_=xr2[h])
                if cw > D:
                    nc.vector.memset(xn2[:, :, :, D:cw], 1.0)
                return xn2

            def zero_out(b):
                ob = ob_pool.tile([NQ, D], f32)
                nc.vector.memset(ob[:], 0.0)
                nc.scalar.dma_start(out=out[b], in_=ob[:])

            skip_dma = stage.startswith("nodma")
            sbase = (
                stage[6:] if stage.startswith("nodma-")
                else ("full" if stage == "nodma" else stage)
            )
            at_dt = bf16 if (mm2 == "mixed" or load == "bf16") else f32

            def transpose_group(xn, xt, g):
                ps_xt = ps_xt_pool.tile(
                    [128, 512], f32 if t_as_mm else xd, tag="ps_xt"
                )
                for j in range(4):
                    c = 4 * g + j
                    dst_ps = ps_xt[:, j * 128 : (j + 1) * 128]
                    if t_as_mm:
                        # regular matmul: out = xn_c^T @ I (HAM-warm + FWL)
                        nc.tensor.matmul(
                            dst_ps, lhsT=xn[:, c, 0:D], rhs=ident[:],
                            start=True, stop=True,
                        )
                        continue
                    for off, w in col_splits(tile_t):
                        kw = {} if w == D else {"tile_position": (0, off)}
                        nc.tensor.transpose(
                            dst_ps[off : off + w, :],
                            xn[:, c, off : off + w],
                            ident[:],
                            **kw,
                        )
                dst = xt[:, g * 512 : (g + 1) * 512]
                if g % 2 == 0:
                    nc.scalar.copy(dst, ps_xt[:])
                else:
                    nc.vector.tensor_copy(dst, ps_xt[:])

            def mm1_chunk(b, xt, ps_qk, c):
                for off, w in col_splits(tile_m1):
                    kw = {} if w == D else {"tile_position": (0, off)}
                    nc.tensor.matmul(
                        ps_qk[off : off + w, c, :],
                        lhsT=xt[:, c * 128 + off : c * 128 + off + w],
                        rhs=qta[:, b, :],
                        start=True,
                        stop=True,
                        **kw,
                    )

            def mm2_chunk(xn, at, ps_o, c):
                lhsT, rhs = at[:, c, :], xn[:, c, :]
                if mm2 == "f32r" and load != "bf16":
                    lhsT, rhs = lhsT.bitcast(f32r), rhs.bitcast(f32r)
                if mm2p:
                    # pack chunks into 3 PE column groups (M=18 << 128);
                    # group j accumulates chunks c % 3 == j at partition 32j.
                    # Only 3 groups: tile_position=(0, 96) hangs cayman.
                    j = c % 3
                    nc.tensor.matmul(
                        ps_o[32 * j : 32 * j + NQ, :],
                        lhsT=lhsT,
                        rhs=rhs,
                        start=(c < 3),
                        stop=(c >= C - 3),
                        tile_position=(0, 32 * j),
                    )
                    return
                nc.tensor.matmul(
                    ps_o[:],
                    lhsT=lhsT,
                    rhs=rhs,
                    start=(c == 0),
                    stop=(c == C - 1),
                )

            def finish(b, ps_o):
                if mm2p:
                    # PSUM -> SBUF (bf16), then sum the 3 column-group
                    # partials with one selection matmul (engines are
                    # lane-aligned, so cross-partition adds need the PE)
                    kr = 64 + NQ
                    pc = sm_pool.tile([128, cw], bf16, tag="pc")
                    if pc_dve:
                        nc.vector.tensor_copy(pc[0:kr, :], ps_o[0:kr, :])
                    else:
                        nc.scalar.copy(pc[0:kr, :], ps_o[0:kr, :])
                    nc.tensor.matmul(
                        ps_o[0:NQ, :],
                        lhsT=selt[0:kr, :],
                        rhs=pc[0:kr, :],
                        start=True,
                        stop=True,
                    )
                    src = ps_o[0:NQ, :]
                else:
                    src = ps_o
                r = sm_pool.tile([NQ, 1], f32, tag="r")
                nc.vector.reciprocal(r[:], src[:, D : D + 1])
                ob = ob_pool.tile([NQ, D], f32)
                nc.scalar.activation(
                    ob[:],
                    src[:, 0:D],
                    mybir.ActivationFunctionType.Copy,
                    scale=r[:],
                )
                nc.scalar.dma_start(out=out[b], in_=ob[:])

            def transpose_group8(xn, xt, g):
                # 8 transposes into one full 2KB bank + one wide copy
                ps_xt = ps_xt_pool.tile([128, 1024], xd, tag="ps_xt")
                for j in range(8):
                    c = 8 * g + j
                    nc.tensor.transpose(
                        ps_xt[:, j * 128 : (j + 1) * 128],
                        xn[:, c, 0:D],
                        ident[:],
                    )
                dst = xt[:, g * 1024 : (g + 1) * 1024]
                if g % 2 == 0:
                    nc.scalar.copy(dst, ps_xt[:])
                else:
                    nc.vector.tensor_copy(dst, ps_xt[:])

            def head(b, xn):
                xt = xt_pool.tile([128, C * 128], bf16, tag="xt")
                if tgroup == 8:
                    for g in range(2):
                        transpose_group8(xn, xt, g)
                else:
                    for g in range(4):
                        transpose_group(xn, xt, g)
                if sbase == "t":
                    return None
                ps_qk = ps_qk_pool.tile([128, C, NQ], f32, tag="ps_qk")
                for c in range(C):
                    mm1_chunk(b, xt, ps_qk, c)
                if sbase == "mm1":
                    return None
                e = e_pool.tile([128, C, NQ], f32, tag="e")
                nc.scalar.activation(
                    e[:], ps_qk[:], mybir.ActivationFunctionType.Exp
                )
                at = e_pool.tile([128, C, NQ], at_dt, tag="at")
                mk_b = mka[:, b, :].unsqueeze(2).broadcast_to([128, C, NQ])
                nc.vector.tensor_mul(at[:], e[:], mk_b)
                return at

            def tail(b, xn, at):
                if sbase in ("t", "mm1"):
                    zero_out(b)
                    return
                ps_o = ps_o_pool.tile(
                    [128 if mm2p else NQ, cw], f32, tag="ps_o"
                )
                for c in range(C):
                    mm2_chunk(xn, at, ps_o, c)
                finish(b, ps_o)

            def body_fine(b, xn):
                # chunk-group-grained pipeline: each 4-chunk group runs
                # transpose -> copy -> MM1 -> exp -> mask -> MM2-accum so
                # the PE never waits a whole batch for the softmax round
                # trip.
                xt = xt_pool.tile([128, C * 128], bf16, tag="xt")
                ps_qk = ps_qk_pool.tile([128, C, NQ], f32, tag="ps_qk")
                e = e_pool.tile([128, C, NQ], f32, tag="e")
                at = e_pool.tile([128, C, NQ], at_dt, tag="at")
                ps_o = ps_o_pool.tile(
                    [128 if mm2p else NQ, cw], f32, tag="ps_o"
                )
                for g in range(4):
                    transpose_group(xn, xt, g)
                    for j in range(4):
                        mm1_chunk(b, xt, ps_qk, 4 * g + j)
                    sl = slice(4 * g, 4 * g + 4)
                    nc.scalar.activation(
                        e[:, sl, :],
                        ps_qk[:, sl, :],
                        mybir.ActivationFunctionType.Exp,
                    )
                    mk_b = (
                        mka[:, b, sl].unsqueeze(2).broadcast_to([128, 4, NQ])
                    )
                    nc.vector.tensor_mul(at[:, sl, :], e[:, sl, :], mk_b)
                    for j in range(4):
                        mm2_chunk(xn, at, ps_o, 4 * g + j)
                finish(b, ps_o)

            def batch_loop():
                if stage == "dma":
                    for b in range(bpc):
                        load_x(b)
                        zero_out(b)
                    return
                if pair:
                    for h in range(bpc // 2):
                        xn2 = load_pair(h)
                        for s2 in range(2):
                            b = 2 * h + s2
                            xn = xn2[:, s2]
                            at = head(b, xn)
                            tail(b, xn, at)
                    return
                if fine:
                    for b in range(bpc):
                        xn = load_x(b)
                        body_fine(b, xn)
                    return
                prev = None
                for b in range(bpc):
                    xn = load_x(b)
                    at = head(b, xn)
                    if not skew:
                        tail(b, xn, at)
                        continue
                    if prev is not None:
                        tail(*prev)
                    prev = (b, xn, at)
                if skew and prev is not None:
                    tail(*prev)

            if reps > 1:
                with tc.For_i(0, reps, 1):
                    batch_loop()
            else:
                batch_loop()

    nc.compile()
    return nc


def build_nc(compute: str = "bf16", bpc: int = BPC, reps: int = 1,
             tile_t: str = "", tile_m1: str = "", stage: str = "full",
             **kw):
    if compute == "v2":
        return build_nc_v2(bpc=bpc, reps=reps, stage=stage, **kw)
    return build_nc_v1(compute, bpc, reps, tile_t, tile_m1, stage)


def build_nc_v1(compute: str = "bf16", bpc: int = BPC, reps: int = 1,
                tile_t: str = "", tile_m1: str = "", stage: str = "full"):
    """Build the per-core bass program. compute in {'f32','bf16'}.

    reps > 1 wraps the whole batch loop in a hardware For_i that redoes the
    same work `reps` times (same data, same output) — benchmarking only.

    tile_t / tile_m1: column-tiling mode for the transposes / QK matmuls:
    "" (single full-width op), "2x64" (two 64-col tiles at col groups 0/64),
    "4x32" (four 32-col tiles — quadrant 3 hangs cayman, do not use).
    Splitting loads the stationary weights through parallel XBUSes.
    """

    def col_splits(mode):
        if mode == "2x64":
            return [(0, 64), (64, 64)]
        if mode == "4x32":
            return [(0, 32), (32, 32), (64, 32), (96, 32)]
        if mode == "3t":
            return [(0, 32), (32, 32), (64, 64)]
        return [(0, 128)]
    dt = mybir.dt.bfloat16 if compute == "bf16" else mybir.dt.float32
    f32 = mybir.dt.float32
    cast_load = compute == "bf16"

    nc = bacc.Bacc("TRN2", target_bir_lowering=False, debug=False)
    xs = nc.dram_tensor("xs", [bpc, S, D], f32, kind="ExternalInput").ap()
    qts = nc.dram_tensor("qts", [bpc, D, NQ], dt, kind="ExternalInput").ap()
    mks = nc.dram_tensor("mks", [bpc, 128, C], dt, kind="ExternalInput").ap()
    out = nc.dram_tensor("out", [bpc, NQ, D], f32, kind="ExternalOutput").ap()

    xr = xs.rearrange("b (p c) d -> b p c d", p=128)

    with tile.TileContext(nc) as tc:
        with ExitStack() as ctx:
            singles = ctx.enter_context(tc.tile_pool(name="singles", bufs=1))
            xn_pool = ctx.enter_context(tc.tile_pool(name="xn", bufs=3))
            xt_pool = ctx.enter_context(tc.tile_pool(name="xt", bufs=2))
            sm_pool = ctx.enter_context(tc.tile_pool(name="sm", bufs=3))
            e_pool = ctx.enter_context(tc.tile_pool(name="e", bufs=2))
            ob_pool = ctx.enter_context(tc.tile_pool(name="ob", bufs=3))
            ps_xt_pool = ctx.enter_context(
                tc.tile_pool(name="ps_xt", bufs=4, space="PSUM")
            )
            ps_qk_pool = ctx.enter_context(
                tc.tile_pool(name="ps_qk", bufs=2, space="PSUM")
            )
            ps_o_pool = ctx.enter_context(
                tc.tile_pool(name="ps_o", bufs=2, space="PSUM")
            )

            ident = singles.tile([128, 128], dt)
            make_identity(nc, ident[:])

            # all batches' qT and mask in one DMA each (tiny)
            qta = singles.tile([D, bpc, NQ], dt)
            nc.sync.dma_start(out=qta[:], in_=qts.rearrange("b p n -> p b n"))
            mka = singles.tile([128, bpc, C], dt)
            nc.sync.dma_start(out=mka[:], in_=mks.rearrange("b p c -> p b c"))

            def body(b):
                # ---- load x[b]: s=16p+c chunk layout, f32->dt cast in DMA
                xn = xn_pool.tile([128, C, CW], dt)
                eng = nc.gpsimd if cast_load else nc.sync
                eng.dma_start(out=xn[:, :, 0:D], in_=xr[b])
                nc.vector.memset(xn[:, :, D : D + 1], 1.0)

                qt = qta[:, b, :]
                mk = mka[:, b, :]

                if stage == "dma":
                    ob = ob_pool.tile([NQ, D], f32)
                    nc.vector.memset(ob[:], 0.0)
                    nc.sync.dma_start(out=out[b], in_=ob[:])
                    return

                # ---- transpose x chunks: xt[d, 16 chunks of 128 s]
                xt = xt_pool.tile([128, C * 128], dt)
                for g in range(4):
                    ps_xt = ps_xt_pool.tile([128, 512], dt)
                    for j in range(4):
                        c = 4 * g + j
                        dst_ps = ps_xt[:, j * 128 : (j + 1) * 128]
                        for off, w in col_splits(tile_t):
                            kw = {} if w == D else {"tile_position": (0, off)}
                            nc.tensor.transpose(
                                dst_ps[off : off + w, :],
                                xn[:, c, off : off + w],
                                ident[:],
                                **kw,
                            )
                    dst = xt[:, g * 512 : (g + 1) * 512]
                    if g % 2 == 0:
                        nc.scalar.copy(dst, ps_xt[:])
                    else:
                        nc.vector.tensor_copy(dst, ps_xt[:])

                if stage == "t":
                    ob = ob_pool.tile([NQ, D], f32)
                    nc.vector.memset(ob[:], 0.0)
                    nc.sync.dma_start(out=out[b], in_=ob[:])
                    return

                # ---- MM1: qkT[s, nq] per chunk (lhsT = xT_c weights)
                ps_qk = ps_qk_pool.tile([128, C, NQ], f32)
                for c in range(C):
                    for off, w in col_splits(tile_m1):
                        kw = {} if w == D else {"tile_position": (0, off)}
                        nc.tensor.matmul(
                            ps_qk[off : off + w, c, :],
                            lhsT=xt[:, c * 128 + off : c * 128 + off + w],
                            rhs=qt,
                            start=True,
                            stop=True,
                            **kw,
                        )

                if stage == "mm1":
                    ob = ob_pool.tile([NQ, D], f32)
                    nc.vector.memset(ob[:], 0.0)
                    nc.sync.dma_start(out=out[b], in_=ob[:])
                    return

                # ---- softmax numerator: exp, then mask (0/1) broadcast
                e = e_pool.tile([128, C, NQ], dt, tag="e")
                nc.scalar.activation(e[:], ps_qk[:], mybir.ActivationFunctionType.Exp)
                at = e_pool.tile([128, C, NQ], dt, tag="at")
                mk_b = mk.unsqueeze(2).broadcast_to([128, C, NQ])
                nc.vector.tensor_mul(at[:], e[:], mk_b)

                # ---- MM2: accumulate attnT_c^T @ [x_c | 1] over chunks
                ps_o = ps_o_pool.tile([NQ, D + 1], f32)
                for c in range(C):
                    nc.tensor.matmul(
                        ps_o[:],
                        lhsT=at[:, c, :],
                        rhs=xn[:, c, 0 : D + 1],
                        start=(c == 0),
                        stop=(c == C - 1),
                    )

                # ---- normalize and store
                r = sm_pool.tile([NQ, 1], f32, tag="r")
                nc.vector.reciprocal(r[:], ps_o[:, D : D + 1])
                ob = ob_pool.tile([NQ, D], f32)
                nc.scalar.activation(
                    ob[:],
                    ps_o[:, 0:D],
                    mybir.ActivationFunctionType.Copy,
                    scale=r[:],
                )
                nc.sync.dma_start(out=out[b], in_=ob[:])

            if reps > 1:
                with tc.For_i(0, reps, 1):
                    for b in range(bpc):
                        body(b)
            else:
                for b in range(bpc):
                    body(b)

    nc.compile()
    return nc


V2_CONFIG = dict(load="bf16", cw=129, skew=0, mm2p=1)


def _get_nc(compute: str = "v2", bpc: int = BPC):
    key = (compute, bpc)
    if key not in _NC_CACHE:
        if compute == "v2":
            _NC_CACHE[key] = build_nc_v2(bpc=bpc, **V2_CONFIG)
        else:
            _NC_CACHE[key] = build_nc(compute, bpc)
    return _NC_CACHE[key]


def prep_inputs(x, q_emb, questions, mask, compute: str = "bf16"):
    """Host-side prep: gather+scale+transpose the tiny q table, reshape mask."""
    q_emb = np.asarray(q_emb, dtype=np.float32)
    questions = np.asarray(questions)
    mask = np.asarray(mask)
    if compute == "v2":
        q_dt, m_dt = ml_dtypes.bfloat16, np.float32
    else:
        np_dt = ml_dtypes.bfloat16 if compute == "bf16" else np.float32
        q_dt = m_dt = np_dt
    scale = 1.0 / math.sqrt(D)
    q = (q_emb * scale)[questions]                          # (B, NQ, D)
    qT = np.ascontiguousarray(q.transpose(0, 2, 1)).astype(q_dt)  # (B, D, NQ)
    mk = np.ascontiguousarray(mask.astype(m_dt).reshape(B, 128, C))  # s = 16p+c
    return qT, mk


def make_sel():
    """Selection matrix summing 3 PE column-group partials: row p
    contributes to output m iff p in {m, 32+m, 64+m}."""
    sel = np.zeros((128, NQ), dtype=ml_dtypes.bfloat16)
    for j in range(3):
        sel[32 * j : 32 * j + NQ, :] += np.eye(NQ, dtype=ml_dtypes.bfloat16)
    return sel


def make_in_maps(inputs, compute: str = "v2"):
    """Shard FULL inputs into per-core in_maps (extra keys are ignored by
    ncs that don't declare them)."""
    qT, mk = prep_inputs(
        inputs["x"], inputs["q_emb"], inputs["questions"], inputs["mask"],
        compute,
    )
    x = np.ascontiguousarray(np.asarray(inputs["x"]), dtype=np.float32)
    sel = make_sel()
    in_maps = []
    for k in range(N_CORES):
        sl = slice(k * BPC, (k + 1) * BPC)
        in_maps.append(
            {"xs": x[sl], "qts": qT[sl], "mks": mk[sl], "sel": sel}
        )
    return in_maps


def kernel(x, q_emb, questions, mask, compute: str = "v2"):
    nc = _get_nc(compute)
    inputs = {"x": x, "q_emb": q_emb, "questions": questions, "mask": mask}
    in_maps = make_in_maps(inputs, compute)
    res = run_bass_kernel_spmd(nc, in_maps, core_ids=list(range(N_CORES)))
    outs = np.concatenate([res.results[k]["out"] for k in range(N_CORES)], axis=0)
    return np.ascontiguousarray(outs, dtype=np.float32)


if __name__ == "__main__":
    rng = np.random.default_rng(0)
    x = rng.standard_normal((B, S, D), dtype=np.float32)
    q_emb = rng.standard_normal((QDIM, D), dtype=np.float32)
    questions = rng.integers(0, QDIM, size=(B, NQ), dtype=np.int32)
    mask = rng.integers(0, 2, size=(B, S), dtype=np.int32)
    out = kernel(x, q_emb, questions, mask)
    print(out.shape, out.dtype)

